# revision 5
# baseline (speedup 1.0000x reference)
"""DequantSiluAndMulQuant Trainium2 kernel.

Reference computation (per token row of x[8192, 22016] int32):
    xf = x * dequant_scale            (fp32)
    tmp = silu(xf[:, :d]) * xf[:, d:]  with d = 11008
    scale = max|tmp| / 127
    q = clip(round(tmp / scale), -128, 127) -> int8
Returns (q[8192, 11008] int8, scale[8192] fp32).

Kernel strategy (8 NeuronCores, token-sharded, no communication):
  Each core gets 1024 tokens. Let s = dequant_scale. We compute
      tmp' = silu(s*gate) * up_int      (the s factor on `up` is deferred)
  which equals tmp/s elementwise, so the int8 quantization
      q = round(tmp' * 127/max|tmp'|)
  is identical, and the output scale is  max|tmp'| * s/127.
  This saves a full dequant pass over `up`:
    - ACT reads int32 gate directly: silu_g = Silu(s * gate)   (1 pass)
    - DVE tensor_tensor mult fp32 x int32: tmp' = silu_g * up  (1 pass)
    - DVE reduce(max, abs) per chunk -> row absmax             (1 pass)
    - ACT Copy(scale=1/(absmax/127)) fp32 -> int8 (exact RNE + saturation
      verified on HW)                                          (1 pass)
  DMA-bound: ~101.5 MB/core at ~358 GB/s HBM => ~285 us roofline.
"""

import numpy as np

NUM_TOKENS = 8192
TWO_D = 22016
D = TWO_D // 2  # 11008
N_CORES = 8
TOK_PER_CORE = NUM_TOKENS // N_CORES  # 1024
P = 128
N_TILES = TOK_PER_CORE // P  # 8
N_CHUNKS = 4
CHUNK = D // N_CHUNKS  # 2752

_cache = {}


def _build(s: float):
    import concourse.bacc as bacc
    import concourse.mybir as mybir
    import concourse.tile as tile

    A = mybir.AluOpType
    F = mybir.ActivationFunctionType

    nc = bacc.Bacc()
    x = nc.dram_tensor("x", [TOK_PER_CORE, TWO_D], mybir.dt.int32, kind="ExternalInput")
    q = nc.dram_tensor("q", [TOK_PER_CORE, D], mybir.dt.int8, kind="ExternalOutput")
    scale = nc.dram_tensor("scale", [TOK_PER_CORE, 1], mybir.dt.float32, kind="ExternalOutput")

    # x row = [gate(0:D) | up(D:2D)]; view as [tile, p, block, chunk, k]
    xv = x[:, :].rearrange(
        "(t p) (b c k) -> t p b c k", p=P, b=2, c=N_CHUNKS
    )

    with tile.TileContext(nc) as tc:
        with (
            tc.tile_pool(name="xc", bufs=3) as x_pool,
            tc.tile_pool(name="silu", bufs=2) as silu_pool,
            tc.tile_pool(name="tmp", bufs=2) as tmp_pool,
            tc.tile_pool(name="qp", bufs=2) as q_pool,
            tc.tile_pool(name="small", bufs=2) as small_pool,
        ):
            for t in range(N_TILES):
                tmp_t = tmp_pool.tile([P, D], mybir.dt.float32)
                colmax = small_pool.tile([P, N_CHUNKS], mybir.dt.float32)
                for c in range(N_CHUNKS):
                    xc = x_pool.tile([P, 2, CHUNK], mybir.dt.int32)
                    nc.sync.dma_start(out=xc, in_=xv[t, :, :, c, :])
                    gate = xc[:, 0, :]
                    up = xc[:, 1, :]
                    silu_c = silu_pool.tile([P, CHUNK], mybir.dt.float32)
                    nc.scalar.activation(silu_c, gate, F.Silu, scale=s)
                    sl = tmp_t[:, c * CHUNK:(c + 1) * CHUNK]
                    nc.vector.tensor_mul(sl, silu_c, up)
                    nc.vector.tensor_reduce(
                        colmax[:, c:c + 1], sl, axis=mybir.AxisListType.X,
                        op=A.max, apply_absolute_value=True,
                    )
                absmax = small_pool.tile([P, 1], mybir.dt.float32)
                nc.vector.tensor_reduce(
                    absmax, colmax, axis=mybir.AxisListType.X, op=A.max,
                )
                # output scale = absmax * s/127 ; quant divisor = absmax/127
                scale_row = small_pool.tile([P, 1], mybir.dt.float32)
                nc.vector.tensor_scalar_mul(scale_row, absmax, float(np.float32(s) / np.float32(127.0)))
                # scale-out on GpSimd (SWDGE): its wait on DVE must not block
                # the SP in-DMA stream or the ACT silu/quant stream
                nc.gpsimd.dma_start(out=scale[t * P:(t + 1) * P, :], in_=scale_row)
                pre = small_pool.tile([P, 1], mybir.dt.float32)
                nc.vector.tensor_scalar_mul(pre, absmax, 1.0 / 127.0)
                inv = small_pool.tile([P, 1], mybir.dt.float32)
                nc.vector.reciprocal(inv, pre)
                # quantize: first half on DVE (tensor_scalar 2x fp32), second
                # half on ACT — halves the critical path and the kernel tail;
                # both casts are exact RNE+saturation (HW-verified)
                qt = q_pool.tile([P, D], mybir.dt.int8)
                H = D // 2
                nc.vector.tensor_scalar_mul(qt[:, :H], tmp_t[:, :H], inv)
                nc.scalar.activation(qt[:, H:], tmp_t[:, H:], F.Copy, scale=inv)
                nc.gpsimd.dma_start(out=q[t * P:(t + 1) * P, :H], in_=qt[:, :H])
                nc.gpsimd.dma_start(out=q[t * P:(t + 1) * P, H:], in_=qt[:, H:])

    nc.compile()
    return nc


def _get_nc(s: float):
    key = float(s)
    if key not in _cache:
        _cache[key] = _build(key)
    return _cache[key]


def _run(x: np.ndarray, s: float, trace: bool = False, trace_kwargs=None):
    from concourse.bass_utils import run_bass_kernel_spmd

    nc = _get_nc(s)
    in_maps = [
        {"x": np.ascontiguousarray(x[i * TOK_PER_CORE:(i + 1) * TOK_PER_CORE])}
        for i in range(N_CORES)
    ]
    out = run_bass_kernel_spmd(
        nc, in_maps, core_ids=list(range(N_CORES)), trace=trace,
        **(trace_kwargs or {}),
    )
    q_full = np.concatenate([r["q"] for r in out.results], axis=0)
    scale_full = np.concatenate(
        [r["scale"].reshape(-1) for r in out.results], axis=0
    ).astype(np.float32)
    return (q_full, scale_full), out


def kernel(x, dequant_scale):
    x = np.asarray(x, dtype=np.int32)
    s = float(np.float32(dequant_scale))
    (q_full, scale_full), _ = _run(x, s, trace=False)
    return q_full, scale_full


# revision 6
# speedup vs baseline: 1.1762x; 1.1762x over previous
"""DequantSiluAndMulQuant Trainium2 kernel.

Reference computation (per token row of x[8192, 22016] int32):
    xf = x * dequant_scale            (fp32)
    tmp = silu(xf[:, :d]) * xf[:, d:]  with d = 11008
    scale = max|tmp| / 127
    q = clip(round(tmp / scale), -128, 127) -> int8
Returns (q[8192, 11008] int8, scale[8192] fp32).

Kernel strategy (8 NeuronCores, token-sharded, no communication):
  Each core gets 1024 tokens. Let s = dequant_scale. We compute
      tmp' = silu(s*gate) * up_int      (the s factor on `up` is deferred)
  which equals tmp/s elementwise, so the int8 quantization
      q = round(tmp' * 127/max|tmp'|)
  is identical, and the output scale is  max|tmp'| * s/127.
  This saves a full dequant pass over `up`:
    - ACT reads int32 gate directly: silu_g = Silu(s * gate)   (1 pass)
    - DVE tensor_tensor mult fp32 x int32: tmp' = silu_g * up  (1 pass)
    - DVE reduce(max, abs) per chunk -> row absmax             (1 pass)
    - ACT Copy(scale=1/(absmax/127)) fp32 -> int8 (exact RNE + saturation
      verified on HW)                                          (1 pass)
  DMA-bound: ~101.5 MB/core at ~358 GB/s HBM => ~285 us roofline.
"""

import numpy as np

NUM_TOKENS = 8192
TWO_D = 22016
D = TWO_D // 2  # 11008
N_CORES = 8
TOK_PER_CORE = NUM_TOKENS // N_CORES  # 1024
P = 128
N_TILES = TOK_PER_CORE // P  # 8
N_CHUNKS = 4
CHUNK = D // N_CHUNKS  # 2752

_cache = {}


def _build(s: float):
    import concourse.bacc as bacc
    import concourse.mybir as mybir
    import concourse.tile as tile

    A = mybir.AluOpType
    F = mybir.ActivationFunctionType

    nc = bacc.Bacc()
    x = nc.dram_tensor("x", [TOK_PER_CORE, TWO_D], mybir.dt.int32, kind="ExternalInput")
    q = nc.dram_tensor("q", [TOK_PER_CORE, D], mybir.dt.int8, kind="ExternalOutput")
    scale = nc.dram_tensor("scale", [TOK_PER_CORE, 1], mybir.dt.float32, kind="ExternalOutput")

    # x row = [gate(0:D) | up(D:2D)]; view as [tile, p, block, chunk, k]
    xv = x[:, :].rearrange(
        "(t p) (b c k) -> t p b c k", p=P, b=2, c=N_CHUNKS
    )

    with tile.TileContext(nc) as tc:
        with (
            tc.tile_pool(name="xc", bufs=3) as x_pool,
            tc.tile_pool(name="silu", bufs=2) as silu_pool,
            tc.tile_pool(name="tmp", bufs=2) as tmp_pool,
            tc.tile_pool(name="qp", bufs=2) as q_pool,
            tc.tile_pool(name="small", bufs=2) as small_pool,
        ):
            for t in range(N_TILES):
                tmp_t = tmp_pool.tile([P, D], mybir.dt.float32)
                colmax = small_pool.tile([P, N_CHUNKS], mybir.dt.float32)
                for c in range(N_CHUNKS):
                    xc = x_pool.tile([P, 2, CHUNK], mybir.dt.int32)
                    nc.sync.dma_start(out=xc, in_=xv[t, :, :, c, :])
                    gate = xc[:, 0, :]
                    up = xc[:, 1, :]
                    silu_c = silu_pool.tile([P, CHUNK], mybir.dt.float32)
                    nc.scalar.activation(silu_c, gate, F.Silu, scale=s)
                    sl = tmp_t[:, c * CHUNK:(c + 1) * CHUNK]
                    nc.vector.tensor_mul(sl, silu_c, up)
                    nc.vector.tensor_reduce(
                        colmax[:, c:c + 1], sl, axis=mybir.AxisListType.X,
                        op=A.max, apply_absolute_value=True,
                    )
                absmax = small_pool.tile([P, 1], mybir.dt.float32)
                nc.vector.tensor_reduce(
                    absmax, colmax, axis=mybir.AxisListType.X, op=A.max,
                )
                # output scale = absmax * s/127 ; quant divisor = absmax/127
                scale_row = small_pool.tile([P, 1], mybir.dt.float32)
                nc.vector.tensor_scalar_mul(scale_row, absmax, float(np.float32(s) / np.float32(127.0)))
                # scale-out on GpSimd (SWDGE): its wait on DVE must not block
                # the SP in-DMA stream or the ACT silu/quant stream
                nc.gpsimd.dma_start(out=scale[t * P:(t + 1) * P, :], in_=scale_row)
                pre = small_pool.tile([P, 1], mybir.dt.float32)
                nc.vector.tensor_scalar_mul(pre, absmax, 1.0 / 127.0)
                inv = small_pool.tile([P, 1], mybir.dt.float32)
                nc.vector.reciprocal(inv, pre)
                # quantize on ACT in chunk-sized pieces (exact RNE+saturation,
                # HW-verified); each piece's q-out issues from ACT right after
                # it — zero-wait triggers that never block other streams
                qt = q_pool.tile([P, D], mybir.dt.int8)
                for c in range(N_CHUNKS):
                    lo, hi = c * CHUNK, (c + 1) * CHUNK
                    nc.scalar.activation(qt[:, lo:hi], tmp_t[:, lo:hi], F.Copy, scale=inv)
                    nc.scalar.dma_start(out=q[t * P:(t + 1) * P, lo:hi], in_=qt[:, lo:hi])

    nc.compile()
    return nc


def _get_nc(s: float):
    key = float(s)
    if key not in _cache:
        _cache[key] = _build(key)
    return _cache[key]


def _run(x: np.ndarray, s: float, trace: bool = False, trace_kwargs=None):
    from concourse.bass_utils import run_bass_kernel_spmd

    nc = _get_nc(s)
    in_maps = [
        {"x": np.ascontiguousarray(x[i * TOK_PER_CORE:(i + 1) * TOK_PER_CORE])}
        for i in range(N_CORES)
    ]
    out = run_bass_kernel_spmd(
        nc, in_maps, core_ids=list(range(N_CORES)), trace=trace,
        **(trace_kwargs or {}),
    )
    q_full = np.concatenate([r["q"] for r in out.results], axis=0)
    scale_full = np.concatenate(
        [r["scale"].reshape(-1) for r in out.results], axis=0
    ).astype(np.float32)
    return (q_full, scale_full), out


def kernel(x, dequant_scale):
    x = np.asarray(x, dtype=np.int32)
    s = float(np.float32(dequant_scale))
    (q_full, scale_full), _ = _run(x, s, trace=False)
    return q_full, scale_full


# revision 7
# speedup vs baseline: 1.2037x; 1.0233x over previous
"""DequantSiluAndMulQuant Trainium2 kernel.

Reference computation (per token row of x[8192, 22016] int32):
    xf = x * dequant_scale            (fp32)
    tmp = silu(xf[:, :d]) * xf[:, d:]  with d = 11008
    scale = max|tmp| / 127
    q = clip(round(tmp / scale), -128, 127) -> int8
Returns (q[8192, 11008] int8, scale[8192] fp32).

Kernel strategy (8 NeuronCores, token-sharded, no communication):
  Each core gets 1024 tokens. Let s = dequant_scale. We compute
      tmp' = silu(s*gate) * up_int      (the s factor on `up` is deferred)
  which equals tmp/s elementwise, so the int8 quantization
      q = round(tmp' * 127/max|tmp'|)
  is identical, and the output scale is  max|tmp'| * s/127.
  This saves a full dequant pass over `up`:
    - ACT reads int32 gate directly: silu_g = Silu(s * gate)   (1 pass)
    - DVE tensor_tensor mult fp32 x int32: tmp' = silu_g * up  (1 pass)
    - DVE reduce(max, abs) per chunk -> row absmax             (1 pass)
    - quantize fp32 -> int8 with per-partition 1/scale (exact RNE +
      saturation on both ACT and DVE, HW-verified)             (1 pass)
  DMA-bound: ~101.5 MB/core at ~358 GB/s HBM => ~284 us roofline.
  In-DMAs issue from SP (HWDGE); q-outs from ACT right after each quant
  piece (zero-wait); the 8 per-tile scales accumulate in SBUF and leave
  in one SP DMA at the end (GpSimd does no DMA => cheap final drain).
  The last tile uses finer chunks and a DVE||ACT quant split to shorten
  the post-last-byte tail.
"""

import numpy as np

NUM_TOKENS = 8192
TWO_D = 22016
D = TWO_D // 2  # 11008
N_CORES = 8
TOK_PER_CORE = NUM_TOKENS // N_CORES  # 1024
P = 128
N_TILES = TOK_PER_CORE // P  # 8

_cache = {}


def _build(s: float):
    import concourse.bacc as bacc
    import concourse.mybir as mybir
    import concourse.tile as tile

    A = mybir.AluOpType
    F = mybir.ActivationFunctionType

    nc = bacc.Bacc()
    x = nc.dram_tensor("x", [TOK_PER_CORE, TWO_D], mybir.dt.int32, kind="ExternalInput")
    q = nc.dram_tensor("q", [TOK_PER_CORE, D], mybir.dt.int8, kind="ExternalOutput")
    scale = nc.dram_tensor("scale", [TOK_PER_CORE, 1], mybir.dt.float32, kind="ExternalOutput")

    # x row = [gate(0:D) | up(D:2D)]; finest view [tile, p, block, cf, kf]
    # with 8 sub-chunks of 1376; coarse chunks take 2 sub-chunks at a time.
    KF = D // 8  # 1376
    xv = x[:, :].rearrange("(t p) (b cf kf) -> t p b cf kf", p=P, b=2, cf=8)

    with tile.TileContext(nc) as tc:
        with (
            tc.tile_pool(name="xc", bufs=3) as x_pool,
            tc.tile_pool(name="silu", bufs=2) as silu_pool,
            tc.tile_pool(name="tmp", bufs=2) as tmp_pool,
            tc.tile_pool(name="qp", bufs=2) as q_pool,
            tc.tile_pool(name="small", bufs=2) as small_pool,
            tc.tile_pool(name="sacc", bufs=1) as sacc_pool,
        ):
            scale_acc = sacc_pool.tile([P, N_TILES], mybir.dt.float32)
            for t in range(N_TILES):
                last = t == N_TILES - 1
                # last tile: finer chunks shorten the post-DMA tail chain
                n_sub = 1 if not last else 2  # sub-chunks per coarse chunk... coarse=2 subs
                chunk_cols = [2 * KF] * 4 if not last else [KF] * 8
                tmp_t = tmp_pool.tile([P, D], mybir.dt.float32)
                colmax = small_pool.tile([P, len(chunk_cols)], mybir.dt.float32)
                col0 = 0
                for ci, cols in enumerate(chunk_cols):
                    cf0 = col0 // KF
                    nsub = cols // KF
                    xc = x_pool.tile([P, 2, cols], mybir.dt.int32, tag="xc")
                    src = xv[t, :, :, cf0:cf0 + nsub, :]
                    nc.sync.dma_start(out=xc, in_=src)
                    gate = xc[:, 0, :]
                    up = xc[:, 1, :]
                    silu_c = silu_pool.tile([P, cols], mybir.dt.float32, tag="silu")
                    nc.scalar.activation(silu_c, gate, F.Silu, scale=s)
                    sl = tmp_t[:, col0:col0 + cols]
                    nc.vector.tensor_mul(sl, silu_c, up)
                    nc.vector.tensor_reduce(
                        colmax[:, ci:ci + 1], sl, axis=mybir.AxisListType.X,
                        op=A.max, apply_absolute_value=True,
                    )
                    col0 += cols
                absmax = small_pool.tile([P, 1], mybir.dt.float32)
                nc.vector.tensor_reduce(
                    absmax, colmax, axis=mybir.AxisListType.X, op=A.max,
                )
                # output scale = absmax * s/127 ; quant divisor = absmax/127
                nc.vector.tensor_scalar_mul(
                    scale_acc[:, t:t + 1], absmax,
                    float(np.float32(s) / np.float32(127.0)),
                )
                pre = small_pool.tile([P, 1], mybir.dt.float32)
                nc.vector.tensor_scalar_mul(pre, absmax, 1.0 / 127.0)
                inv = small_pool.tile([P, 1], mybir.dt.float32)
                nc.vector.reciprocal(inv, pre)
                qt = q_pool.tile([P, D], mybir.dt.int8)
                if not last:
                    # quantize on ACT in 4 pieces; q-out from ACT after each
                    for c in range(4):
                        lo, hi = c * 2 * KF, (c + 1) * 2 * KF
                        nc.scalar.activation(qt[:, lo:hi], tmp_t[:, lo:hi], F.Copy, scale=inv)
                        nc.scalar.dma_start(out=q[t * P:(t + 1) * P, lo:hi], in_=qt[:, lo:hi])
                else:
                    # tail: DVE does the first half in one 2x tensor_scalar
                    # while ACT does the second half in 2 pieces
                    H = D // 2
                    nc.vector.tensor_scalar_mul(qt[:, :H], tmp_t[:, :H], inv)
                    nc.scalar.activation(qt[:, H:H + 2 * KF], tmp_t[:, H:H + 2 * KF], F.Copy, scale=inv)
                    nc.scalar.dma_start(out=q[t * P:(t + 1) * P, H:H + 2 * KF], in_=qt[:, H:H + 2 * KF])
                    nc.scalar.activation(qt[:, H + 2 * KF:], tmp_t[:, H + 2 * KF:], F.Copy, scale=inv)
                    nc.scalar.dma_start(out=q[t * P:(t + 1) * P, H + 2 * KF:], in_=qt[:, H + 2 * KF:])
                    nc.scalar.dma_start(out=q[t * P:(t + 1) * P, :H], in_=qt[:, :H])
            # one 4KB scale DMA at the very end (SP is idle by then);
            # dst element (p, t) lives at row t*128+p
            sdst = scale[:, :].rearrange("(t p) o -> p (t o)", p=P)
            nc.sync.dma_start(out=sdst, in_=scale_acc)

    nc.compile()
    return nc


def _get_nc(s: float):
    key = float(s)
    if key not in _cache:
        _cache[key] = _build(key)
    return _cache[key]


def _run(x: np.ndarray, s: float, trace: bool = False, trace_kwargs=None):
    from concourse.bass_utils import run_bass_kernel_spmd

    nc = _get_nc(s)
    in_maps = [
        {"x": np.ascontiguousarray(x[i * TOK_PER_CORE:(i + 1) * TOK_PER_CORE])}
        for i in range(N_CORES)
    ]
    out = run_bass_kernel_spmd(
        nc, in_maps, core_ids=list(range(N_CORES)), trace=trace,
        **(trace_kwargs or {}),
    )
    q_full = np.concatenate([r["q"] for r in out.results], axis=0)
    scale_full = np.concatenate(
        [r["scale"].reshape(-1) for r in out.results], axis=0
    ).astype(np.float32)
    return (q_full, scale_full), out


def kernel(x, dequant_scale):
    x = np.asarray(x, dtype=np.int32)
    s = float(np.float32(dequant_scale))
    (q_full, scale_full), _ = _run(x, s, trace=False)
    return q_full, scale_full


# revision 9
# speedup vs baseline: 1.2316x; 1.0232x over previous
"""DequantSiluAndMulQuant Trainium2 kernel.

Reference computation (per token row of x[8192, 22016] int32):
    xf = x * dequant_scale            (fp32)
    tmp = silu(xf[:, :d]) * xf[:, d:]  with d = 11008
    scale = max|tmp| / 127
    q = clip(round(tmp / scale), -128, 127) -> int8
Returns (q[8192, 11008] int8, scale[8192] fp32).

Kernel strategy (8 NeuronCores, token-sharded, no communication):
  Each core gets 1024 tokens. Let s = dequant_scale. We compute
      tmp' = silu(s*gate) * up_int      (the s factor on `up` is deferred)
  which equals tmp/s elementwise, so the int8 quantization
      q = round(tmp' * 127/max|tmp'|)
  is identical, and the output scale is  max|tmp'| * s/127.
  This saves a full dequant pass over `up`:
    - ACT reads int32 gate directly: silu_g = Silu(s * gate)   (1 pass)
    - DVE tensor_tensor mult fp32 x int32: tmp' = silu_g * up  (1 pass)
    - DVE reduce(max, abs) per chunk -> row absmax             (1 pass)
    - quantize fp32 -> int8 with per-partition 1/scale (exact RNE +
      saturation on both ACT and DVE, HW-verified)             (1 pass)
  DMA-bound: ~101.5 MB/core at ~358 GB/s HBM => ~284 us roofline.
  In-DMAs issue from SP (HWDGE); q-outs from ACT right after each quant
  piece (zero-wait); the 8 per-tile scales accumulate in SBUF and leave
  in one SP DMA at the end (GpSimd does no DMA => cheap final drain).
  The last tile uses finer chunks and a DVE||ACT quant split to shorten
  the post-last-byte tail.
"""

import numpy as np

NUM_TOKENS = 8192
TWO_D = 22016
D = TWO_D // 2  # 11008
N_CORES = 8
TOK_PER_CORE = NUM_TOKENS // N_CORES  # 1024
P = 128
N_TILES = TOK_PER_CORE // P  # 8

_cache = {}


def _build(s: float):
    import concourse.bacc as bacc
    import concourse.mybir as mybir
    import concourse.tile as tile

    A = mybir.AluOpType
    F = mybir.ActivationFunctionType

    nc = bacc.Bacc()
    x = nc.dram_tensor("x", [TOK_PER_CORE, TWO_D], mybir.dt.int32, kind="ExternalInput")
    q = nc.dram_tensor("q", [TOK_PER_CORE, D], mybir.dt.int8, kind="ExternalOutput")
    scale = nc.dram_tensor("scale", [TOK_PER_CORE, 1], mybir.dt.float32, kind="ExternalOutput")

    # x row = [gate(0:D) | up(D:2D)]; finest view [tile, p, block, cf, kf]
    # with 8 sub-chunks of 1376; coarse chunks take 2 sub-chunks at a time.
    KF = D // 8  # 1376
    xv = x[:, :].rearrange("(t p) (b cf kf) -> t p b cf kf", p=P, b=2, cf=8)

    with tile.TileContext(nc) as tc:
        with (
            tc.tile_pool(name="xc", bufs=3) as x_pool,
            tc.tile_pool(name="silu", bufs=2) as silu_pool,
            tc.tile_pool(name="tmp", bufs=2) as tmp_pool,
            tc.tile_pool(name="qp", bufs=2) as q_pool,
            tc.tile_pool(name="small", bufs=2) as small_pool,
            tc.tile_pool(name="sacc", bufs=1) as sacc_pool,
        ):
            scale_acc = sacc_pool.tile([P, N_TILES], mybir.dt.float32)
            tmp_tiles = {}
            inv_tiles = {}

            def emit_quant_piece(tp, c):
                """Quantize piece c of tile tp on ACT + q-out (zero-wait)."""
                lo, hi = c * 2 * KF, (c + 1) * 2 * KF
                qt = q_pool.tile([P, 2 * KF], mybir.dt.int8, tag="qt")
                nc.scalar.activation(qt, tmp_tiles[tp][:, lo:hi], F.Copy, scale=inv_tiles[tp])
                nc.scalar.dma_start(out=q[tp * P:(tp + 1) * P, lo:hi], in_=qt)

            for t in range(N_TILES):
                last = t == N_TILES - 1
                # last tile: finer chunks shorten the post-DMA tail chain
                chunk_cols = [2 * KF] * 4 if not last else [KF] * 8
                tmp_t = tmp_pool.tile([P, D], mybir.dt.float32, tag="tmp")
                tmp_tiles[t] = tmp_t
                colmax = small_pool.tile([P, len(chunk_cols)], mybir.dt.float32, tag="colmax")
                col0 = 0
                for ci, cols in enumerate(chunk_cols):
                    cf0 = col0 // KF
                    nsub = cols // KF
                    xc = x_pool.tile([P, 2, cols], mybir.dt.int32, tag="xc")
                    src = xv[t, :, :, cf0:cf0 + nsub, :]
                    nc.sync.dma_start(out=xc, in_=src)
                    gate = xc[:, 0, :]
                    up = xc[:, 1, :]
                    silu_c = silu_pool.tile([P, cols], mybir.dt.float32, tag="silu")
                    nc.scalar.activation(silu_c, gate, F.Silu, scale=s)
                    sl = tmp_t[:, col0:col0 + cols]
                    nc.vector.tensor_mul(sl, silu_c, up)
                    nc.vector.tensor_reduce(
                        colmax[:, ci:ci + 1], sl, axis=mybir.AxisListType.X,
                        op=A.max, apply_absolute_value=True,
                    )
                    col0 += cols
                    # software pipeline: previous tile's quant pieces slot
                    # between this tile's silu chunks on ACT, so quant never
                    # blocks the silu stream at a tile boundary
                    if t >= 1 and not last and ci < 4:
                        emit_quant_piece(t - 1, ci)
                    elif last and ci % 2 == 1 and ci < 8:
                        emit_quant_piece(t - 1, ci // 2)
                absmax = small_pool.tile([P, 1], mybir.dt.float32, tag="absmax")
                nc.vector.tensor_reduce(
                    absmax, colmax, axis=mybir.AxisListType.X, op=A.max,
                )
                # output scale = absmax * s/127 ; quant divisor = absmax/127
                nc.vector.tensor_scalar_mul(
                    scale_acc[:, t:t + 1], absmax,
                    float(np.float32(s) / np.float32(127.0)),
                )
                pre = small_pool.tile([P, 1], mybir.dt.float32, tag="pre")
                nc.vector.tensor_scalar_mul(pre, absmax, 1.0 / 127.0)
                inv = small_pool.tile([P, 1], mybir.dt.float32, tag="inv")
                nc.vector.reciprocal(inv, pre)
                inv_tiles[t] = inv

            # tail: last tile quantizes immediately — DVE takes the first
            # half in one 2x tensor_scalar while ACT does the second half
            t = N_TILES - 1
            tmp_t, inv = tmp_tiles[t], inv_tiles[t]
            H = D // 2
            qt_l = sacc_pool.tile([P, D], mybir.dt.int8, tag="qt_last")
            nc.vector.tensor_scalar_mul(qt_l[:, :H], tmp_t[:, :H], inv)
            nc.scalar.activation(qt_l[:, H:H + 2 * KF], tmp_t[:, H:H + 2 * KF], F.Copy, scale=inv)
            nc.scalar.dma_start(out=q[t * P:(t + 1) * P, H:H + 2 * KF], in_=qt_l[:, H:H + 2 * KF])
            nc.scalar.activation(qt_l[:, H + 2 * KF:], tmp_t[:, H + 2 * KF:], F.Copy, scale=inv)
            nc.scalar.dma_start(out=q[t * P:(t + 1) * P, H + 2 * KF:], in_=qt_l[:, H + 2 * KF:])
            nc.scalar.dma_start(out=q[t * P:(t + 1) * P, :H], in_=qt_l[:, :H])
            # one 4KB scale DMA at the very end (SP is idle by then);
            # dst element (p, t) lives at row t*128+p
            sdst = scale[:, :].rearrange("(t p) o -> p (t o)", p=P)
            nc.sync.dma_start(out=sdst, in_=scale_acc)

    nc.compile()
    return nc


def _get_nc(s: float):
    key = float(s)
    if key not in _cache:
        _cache[key] = _build(key)
    return _cache[key]


def _run(x: np.ndarray, s: float, trace: bool = False, trace_kwargs=None):
    from concourse.bass_utils import run_bass_kernel_spmd

    nc = _get_nc(s)
    in_maps = [
        {"x": np.ascontiguousarray(x[i * TOK_PER_CORE:(i + 1) * TOK_PER_CORE])}
        for i in range(N_CORES)
    ]
    out = run_bass_kernel_spmd(
        nc, in_maps, core_ids=list(range(N_CORES)), trace=trace,
        **(trace_kwargs or {}),
    )
    q_full = np.concatenate([r["q"] for r in out.results], axis=0)
    scale_full = np.concatenate(
        [r["scale"].reshape(-1) for r in out.results], axis=0
    ).astype(np.float32)
    return (q_full, scale_full), out


def kernel(x, dequant_scale):
    x = np.asarray(x, dtype=np.int32)
    s = float(np.float32(dequant_scale))
    (q_full, scale_full), _ = _run(x, s, trace=False)
    return q_full, scale_full
